# revision 1
# baseline (speedup 1.0000x reference)
"""Bilinear LTN scoring kernel for Trainium2 (8 NeuronCores).

scores[i] = ent[h[i]]^T @ W[r[i]] @ ent[t[i]],  B=4096, DIM=256.

Strategy: items grouped by relation (<=32 per group); groups sharded
across the 8 cores so each relation matrix streams from HBM once
system-wide. Per core ~63 groups -> ~8MB fp16 W stream; the kernel is
HBM-bound on that stream and everything else hides under it.

Layout (per core; the *structure* is identical across cores, only the
data differs — SPMD shares one program):
  - group g owns grid slots [32g, 32g+32); grid block m = groups
    [4m, 4m+4) = one [128, 256] PSUM tile (4 stationary strips of 32).
  - groups are packed into "windows" of <=12 groups / <=128 items.
    Window w owns grid blocks [3w, 3w+3) (the last window may own
    fewer). Each window's items live compactly in one 128-row gather
    block, so h and t need only one [128,1]-offset indirect DMA per
    window (the only indirect-DMA shape this HW supports; multi-column
    offset APs were probed and gather consecutive rows instead).
  - per (window, k-half): ONE PE matmul transposes+expands compact H
    into grid layout: htp[a, s] = sum_c hc[c, 128k+a] * P[c, s]
    (P one-hot, [128, 384], streamed from DRAM).
  - per block: one PE matmul expands compact t into grid rows:
    texp[s, :] = sum_c P[c, s] tc[c, :] (fp16 in, fp32 PSUM out).
  - W host-packed [128, G*512] so each window's chunk DMA moves
    contiguous partition lines; all tiles resident in SBUF (no
    recycling), issued up front on both HWDGE queues.
  - per block: 8 matmuls (2 contract halves x 4 strips) accumulate
    h^T W in PSUM; then mult by texp + reduce -> 128 scores.
  - pad slots gather real scattered ent rows (NOT skipped): stale SBUF
    in unwritten pad rows would otherwise feed 0*NaN into the P
    matmuls and contaminate every expansion row. Their results land in
    unused output rows.
  - 44 PE warm-up matmuls keep the HAM clock gate up until gather data
    arrives (cold/mid p-state makes every matmul ~2.3x slower).
"""

import sys

for _p in ("/opt/trn_rl_repo",):
    if _p not in sys.path:
        sys.path.insert(0, _p)

import ml_dtypes
import numpy as np

import concourse.bass as bass
import concourse.mybir as mybir
import concourse.tile as tile
from concourse.bass import IndirectOffsetOnAxis
from concourse.bass_utils import run_bass_kernel_spmd
from concourse.vector_clock import ScopedClock

DIM = 256
N_ENT = 100000
N_REL = 500
NCORES = 8
C = 32                 # grid slots per group (matmul stationary width)
WGROUPS = 12           # groups per window (3 grid blocks)
WSLOTS = 3 * 128       # grid slots per window
PAD_IDX = 0x7FFF0000   # > N_ENT-1 -> indirect DMA skips the row

F32 = mybir.dt.float32
FP16 = mybir.dt.float16
I32 = mybir.dt.int32

MODE = "fp16"  # kept for test.py compatibility

# W stream dtype: False = fp16; True = float8_e3m4 with per-group scale
# (dequantized on the host at extraction time).
W8 = False
W8_TARGET = 14.0  # scale absmax of each W to this (e3m4 max is 15.5)

# CoreSim-only: memset gather buffers so the interpreter's uninitialized-
# read checks pass (pad slots are never written by the gathers).
SIM_INIT = False

_MAX_WAITS = 1


def _install_walrus_fixes():
    """This container's walrus accepts only one sync wait per instruction;
    split extra waits onto preceding same-engine NOPs."""
    if getattr(tile.TileContext, "_drain_fix_installed", False):
        return

    def _split_multi_waits(nc):
        cur_bb = nc.cur_bb.bb
        for f in nc.m.functions:
            for blk in f.blocks:
                bb = blk if hasattr(blk, "instructions") else blk.bb
                i = 0
                while i < len(bb.instructions):
                    inst = bb.instructions[i]
                    si = getattr(inst, "sync_info", None)
                    waits = list(si.on_wait or []) if si is not None else []
                    if len(waits) > _MAX_WAITS:
                        si.on_wait = waits[-_MAX_WAITS:]
                        extra = waits[: -_MAX_WAITS]
                        nops = []
                        for w0 in range(0, len(extra), _MAX_WAITS):
                            nop_inst = nc.engines[inst.engine].nop(
                                nofuse=True, hint="wait_split"
                            )
                            nop_inst.ins.sync_info = mybir.SyncInfo(
                                on_wait=extra[w0 : w0 + _MAX_WAITS],
                                on_update=[],
                            )
                            nops.append(nop_inst.ins)
                        for n in nops:
                            cur_bb.instructions.remove(n)
                        for j, n in enumerate(nops):
                            bb.instructions.insert(i + j, n)
                        i += len(nops)
                    i += 1

    def _drain_and_barrier(self, tick_clock, wait_clock):
        drain_inst = self.nc.sync.drain()
        wait_clock.add_sem_waits(
            drain_inst.ins, ScopedClock({None: tick_clock.global_clock})
        )
        self.nc.all_engine_barrier()
        assert self.sems is not None
        popped = self.nc._tile_sem_poison_stack.pop()
        assert popped is self._sem_poison
        self.nc.clear_and_free_semaphores(list(self.sems.allocated().values()))
        self.nc.all_engine_barrier()
        _split_multi_waits(self.nc)

    tile.TileContext._drain_and_barrier = _drain_and_barrier
    tile.TileContext._drain_fix_installed = True


def _wstride(comp):
    # pmat row stride = widest window's grid slots (384 for uniform windows)
    return max(comp) * 128


def _wcomp(NBLK, first=3):
    """Window block-composition. first=1 gives [1,3,3,...]: the first
    window's W chunk is only 512KB, so the DMA-engine FIFOs hold minimal
    W backlog ahead of the earliest gathers and the pipeline fills sooner.
    first=3 is the uniform layout; first=4 was HW-measured slower."""
    comp = []
    left = NBLK
    while left > 0:
        b = min(first if not comp else 3, left)
        comp.append(b)
        left -= b
    return comp


def _build_bass(G, first=3, w8=None):
    _install_walrus_fixes()
    if w8 is None:
        w8 = W8
    WDT = mybir.dt.float8e3 if w8 else FP16
    PDT = mybir.dt.float8e3 if w8 else FP16
    NBLK = (G + 3) // 4
    comp = _wcomp(NBLK, first)
    NWIN = len(comp)
    bbase = [0]
    for b in comp:
        bbase.append(bbase[-1] + b)
    blk2win = [None] * NBLK
    for w in range(NWIN):
        for j in range(bbase[w], bbase[w + 1]):
            blk2win[j] = w
    WSTR = _wstride(comp)

    nc = bass.Bass()
    ent16 = nc.declare_dram_parameter("ent16", [N_ENT, DIM], FP16, isOutput=False)
    wpk = nc.declare_dram_parameter("wpk", [128, G * 512], WDT, isOutput=False)
    idx = nc.declare_dram_parameter("idx", [128, 2 * NWIN], I32, isOutput=False)
    pmat = nc.declare_dram_parameter(
        "pmat", [NWIN, 128 * WSTR], PDT, isOutput=False
    )
    out = nc.declare_dram_parameter("out", [128, NBLK], F32, isOutput=True)

    with tile.TileContext(nc) as tc:
        with (
            tc.tile_pool(name="const", bufs=1) as const_pool,
            tc.tile_pool(name="w", bufs=1) as w_pool,
            tc.tile_pool(name="sc", bufs=3) as sc_pool,
            tc.tile_pool(name="te", bufs=4) as te_sb_pool,
            tc.tile_pool(name="upsum", bufs=3, space="PSUM") as u_pool,
            tc.tile_pool(name="tepsum", bufs=3, space="PSUM") as te_pool,
            tc.tile_pool(name="trpsum", bufs=2, space="PSUM") as tr_pool,
        ):
            # ---- PE warm-up: dense dummy matmuls, no data deps
            dummy = const_pool.tile([128, DIM], FP16, tag="dummy")
            nc.vector.memset(dummy[:], 0.0)
            dps = u_pool.tile([128, DIM], F32, space="PSUM", tag="ups", name="dps")
            for wu in range(48):
                nc.tensor.matmul(
                    out=dps[0:32, :],
                    lhsT=dummy[:, 0:32],
                    rhs=dummy[:],
                    start=True,
                    stop=True,
                    tile_position=(0, 0),
                )

            # idx first on sync so the gather chain starts ASAP
            idx_t = const_pool.tile([128, 2 * NWIN], I32, tag="idx")
            nc.sync.dma_start(out=idx_t[:], in_=idx[:])
            # P blocks: pt[:, w*WSTR + s] = P_w[c, s]
            pt = const_pool.tile([128, NWIN * WSTR], PDT, tag="pt")
            nc.scalar.dma_start(
                out=pt[:].rearrange("c (w s) -> c w s", w=NWIN, s=WSTR),
                in_=pmat[:].rearrange("w (c s) -> c w s", c=128, s=WSTR),
            )
            # pay ACT's one-time activation-table load before the queue
            # gets busy (it otherwise lands on the critical tail)
            actwarm = const_pool.tile([128, 1], F32, tag="actwarm")
            nc.vector.memset(actwarm[:], 0.0)
            nc.scalar.copy(actwarm[:], actwarm[:])

            out_sb = const_pool.tile([128, NBLK], F32, tag="outsb")
            nc.vector.memset(out_sb[:], 0.0)
            hcs = [
                const_pool.tile([128, DIM], FP16, tag=f"hc{w}", name=f"hc{w}")
                for w in range(NWIN)
            ]
            tcs = [
                const_pool.tile([128, DIM], FP16, tag=f"tc{w}", name=f"tc{w}")
                for w in range(NWIN)
            ]
            if SIM_INIT:
                for w in range(NWIN):
                    nc.vector.memset(hcs[w][:], 0.0)
                    nc.vector.memset(tcs[w][:], 0.0)

            # ---- gathers + window-paced W stream
            # One [128,1]-offset indirect DMA per (side, window). Each
            # window's W-chunk DMA is gated on a tiny pool-queue memset
            # emitted right after that window's h gather: this paces the W
            # descriptors into the (FIFO-drained) DMA engines in consumption
            # order, so a window's gathers never queue behind later
            # windows' megabyte W transfers.
            # ---- gathers + W stream (W in per-window chunks, both queues)
            wts = {}  # block m -> (tile, col offset)
            for w in range(NWIN):
                b0 = bbase[w]
                nb = comp[w]
                nc.gpsimd.indirect_dma_start(
                    out=hcs[w][:],
                    out_offset=None,
                    in_=ent16[:],
                    in_offset=IndirectOffsetOnAxis(ap=idx_t[:, w : w + 1], axis=0),
                )
                if nb > 0:
                    ncols = min(G * 512, (b0 + nb) * 2048) - b0 * 2048
                    wt = w_pool.tile(
                        [128, nb * 2048], WDT, tag=f"wt{w}", name=f"wt{w}"
                    )
                    for m in range(b0, b0 + nb):
                        wts[m] = (wt, (m - b0) * 2048)
                    eng = nc.sync if w % 2 == 0 else nc.scalar
                    eng.dma_start(
                        out=wt[:, :ncols],
                        in_=wpk[:, b0 * 2048 : b0 * 2048 + ncols],
                    )
                nc.gpsimd.indirect_dma_start(
                    out=tcs[w][:],
                    out_offset=None,
                    in_=ent16[:],
                    in_offset=IndirectOffsetOnAxis(
                        ap=idx_t[:, NWIN + w : NWIN + w + 1], axis=0
                    ),
                )

            # ---- per-window: H transpose+expand, then that window's blocks
            # (window-interleaved emission keeps each engine queue's head
            # blocked only on its OWN window's gather)
            htps = {}

            def _emit_htrp(w):
                # htp_w_k[a, s] = sum_c hc[c, 128k + a] * P_w[c, s]
                ws = comp[w] * 128
                for k in range(2):
                    trp = tr_pool.tile(
                        [128, ws], F32, space="PSUM", tag="trp",
                        name=f"trp{w}_{k}",
                    )
                    nc.tensor.matmul(
                        out=trp[:],
                        lhsT=hcs[w][:, k * 128 : (k + 1) * 128],
                        rhs=pt[:, w * WSTR : w * WSTR + ws],
                        start=True,
                        stop=True,
                    )
                    htp = const_pool.tile(
                        [128, ws], FP16, tag=f"htp{w}_{k}", name=f"htp{w}_{k}"
                    )
                    if k == 0:
                        nc.vector.tensor_copy(htp[:], trp[:])
                    else:
                        nc.scalar.copy(htp[:], trp[:])
                    htps[(w, k)] = htp

            for m in range(NBLK):
                nstrip = min(G, 4 * m + 4) - 4 * m
                w = blk2win[m]
                jloc = m - bbase[w]
                npart = 32 * nstrip
                if (w, 0) not in htps:
                    _emit_htrp(w)

                texp = te_pool.tile(
                    [128, DIM], F32, space="PSUM", tag="texp", name=f"texp{m}"
                )
                nc.tensor.matmul(
                    out=texp[0:npart, :],
                    lhsT=pt[
                        :, w * WSTR + jloc * 128 : w * WSTR + jloc * 128 + npart
                    ],
                    rhs=tcs[w][:],
                    start=True,
                    stop=True,
                )
                texp_sb = te_sb_pool.tile(
                    [128, DIM], F32, tag="tesb", name=f"tesb{m}"
                )
                nc.scalar.copy(texp_sb[0:npart, :], texp[0:npart, :])

                ups = u_pool.tile(
                    [128, DIM], F32, space="PSUM", tag="ups", name=f"ups{m}"
                )
                wtile, wcol = wts[m]
                for k in range(2):
                    for d in range(nstrip):
                        nc.tensor.matmul(
                            out=ups[32 * d : 32 * d + 32, :],
                            lhsT=htps[(w, k)][
                                :, jloc * 128 + 32 * d : jloc * 128 + 32 * d + 32
                            ],
                            rhs=wtile[
                                :,
                                wcol
                                + (d * 2 + k) * DIM : wcol
                                + (d * 2 + k + 1) * DIM,
                            ],
                            start=(k == 0),
                            stop=(k == 1),
                            tile_position=(0, 32 * d),
                            skip_group_check=True,
                        )

                sc = sc_pool.tile([128, DIM], F32, tag="sc", name=f"sc{m}")
                nc.vector.tensor_tensor(
                    out=sc[0:npart, :],
                    in0=ups[0:npart, :],
                    in1=texp_sb[0:npart, :],
                    op=mybir.AluOpType.mult,
                )
                nc.vector.tensor_reduce(
                    out=out_sb[0:npart, m : m + 1],
                    in_=sc[0:npart, :],
                    axis=mybir.AxisListType.X,
                    op=mybir.AluOpType.add,
                )
                if m == 11 and NBLK > 12:
                    # early partial writeback: the final DMA then waits only
                    # on the last blocks' reduces
                    nc.sync.dma_start(
                        out=out[:, 0:12], in_=out_sb[:, 0:12]
                    )

            if NBLK > 12:
                nc.sync.dma_start(out=out[:, 12:], in_=out_sb[:, 12:])
            else:
                nc.sync.dma_start(out=out[:], in_=out_sb[:])

    return nc


_NC_CACHE = {}


def _get_nc(G, first):
    key = (G, first, W8)
    if key not in _NC_CACHE:
        _NC_CACHE[key] = _build_bass(G, first)
    return _NC_CACHE[key]


def _pack(h, r, t, rel_weight, first=3):
    """Group items by relation (chunks <=C), balance chunks across cores,
    pack each core's chunks into windows (block composition from _wcomp;
    every window fills its group-slot quota and holds <=128 items)."""
    order = np.argsort(r, kind="stable")
    rs = r[order]
    starts = np.flatnonzero(np.r_[True, rs[1:] != rs[:-1]])
    ends = np.r_[starts[1:], len(rs)]
    chunks = []  # (rel_id, item_positions)
    for s0, e0 in zip(starts, ends):
        rid = int(rs[s0])
        for c0 in range(s0, e0, C):
            chunks.append((rid, order[c0 : min(c0 + C, e0)]))
    chunks.sort(key=lambda x: -len(x[1]))

    per_core = [[] for _ in range(NCORES)]
    counts = [0] * NCORES
    items = [0] * NCORES
    for ch in chunks:
        k = min(range(NCORES), key=lambda q: (counts[q], items[q]))
        per_core[k].append(ch)
        counts[k] += 1
        items[k] += len(ch[1])

    G = max(counts)
    NBLK = (G + 3) // 4
    comp = _wcomp(NBLK, first)
    NWIN = len(comp)
    bbase = [0]
    for b in comp:
        bbase.append(bbase[-1] + b)
    # window wi holds exactly quota[wi] group slots (last takes remainder)
    quota = [min(4 * comp[wi], G - 4 * bbase[wi]) for wi in range(NWIN)]
    WSTR = _wstride(comp)

    wdt = ml_dtypes.float8_e3m4 if W8 else np.float16
    in_maps = []
    slot_maps = []
    for k in range(NCORES):
        # assign chunks to windows (greedy on item counts, sorted desc;
        # every window must fill to its quota and stay <=128 items)
        wins = [[] for _ in range(NWIN)]
        witems = [0] * NWIN
        for ch in per_core[k]:
            n = len(ch[1])
            cand = [
                wi
                for wi in range(NWIN)
                if len(wins[wi]) < quota[wi] and witems[wi] + n <= 128
            ]
            if not cand:  # fall through; the overflow assert fires below
                cand = [
                    wi for wi in range(NWIN) if len(wins[wi]) < quota[wi]
                ]
            wi = min(cand, key=lambda q: witems[q])
            wins[wi].append(ch)
            witems[wi] += n
        assert max(witems) <= 128, f"window overflow: {witems}"

        wpk = np.zeros((128, G * 512), dtype=wdt)
        # pad slots gather real (distinct) rows, NOT skipped: stale SBUF in
        # pad rows would otherwise inject NaN through the 0*NaN terms of the
        # P matmuls; distinct rows avoid same-address HBM hotspots
        padrows = (
            np.arange(128 * NWIN, dtype=np.int32).reshape(128, NWIN) * 37
        ) % N_ENT
        hidx = padrows.copy()
        tidx = padrows.copy()
        pmat = np.zeros((NWIN, 128, WSTR), dtype=np.float16)
        slots = []
        positions = []
        descale = []
        for wi in range(NWIN):
            cpos = 0
            for gl, (rid, pos) in enumerate(wins[wi]):
                g = 4 * bbase[wi] + gl
                w3 = (
                    rel_weight[rid]
                    .reshape(2, 128, DIM)
                    .transpose(1, 0, 2)
                    .reshape(128, 512)
                )
                if W8:
                    sg = W8_TARGET / max(np.abs(w3).max(), 1e-30)
                    wpk[:, g * 512 : (g + 1) * 512] = (w3 * sg).astype(wdt)
                    descale.append(
                        np.full(len(pos), 1.0 / sg, dtype=np.float64)
                    )
                else:
                    wpk[:, g * 512 : (g + 1) * 512] = w3.astype(wdt)
                s = g * C + np.arange(len(pos))
                cslot = cpos + np.arange(len(pos))
                cpos += len(pos)
                hidx[cslot, wi] = h[pos]
                tidx[cslot, wi] = t[pos]
                # window wi owns grid slots from 128*bbase[wi]
                local_s = s - 128 * bbase[wi]
                pmat[wi, cslot, local_s] = 1.0
                slots.append(s)
                positions.append(pos)
        slots = np.concatenate(slots) if slots else np.zeros(0, np.int64)
        positions = (
            np.concatenate(positions) if positions else np.zeros(0, np.int64)
        )
        descale = np.concatenate(descale) if descale else None
        slot_maps.append((slots, positions, descale))
        in_maps.append(
            {
                "wpk": wpk,
                "idx": np.concatenate([hidx, tidx], axis=1),
                "pmat": pmat.reshape(NWIN, 128 * WSTR),
            }
        )
    return in_maps, slot_maps, G


def _run(h, r, t, ent_weight, rel_weight, trace=False, mode=None):
    h = np.asarray(h).astype(np.int64)
    r = np.asarray(r).astype(np.int64)
    t = np.asarray(t).astype(np.int64)
    ent_weight = np.ascontiguousarray(np.asarray(ent_weight, dtype=np.float32))
    rel_weight = np.ascontiguousarray(np.asarray(rel_weight, dtype=np.float32))
    assert ent_weight.shape == (N_ENT, DIM)
    assert rel_weight.shape == (N_REL, DIM * DIM)

    # uniform 3-block windows: HW-measured fastest (first=4 front-loads a
    # 2MB W chunk ahead of the gathers; first=1 adds per-window overhead
    # without shortening the gather chain — both regressed)
    first = 3
    in_maps, slot_maps, G = _pack(h, r, t, rel_weight, first=3)
    ent16 = ent_weight.astype(np.float16)
    for im in in_maps:
        im["ent16"] = ent16
    nc = _get_nc(G, first)
    res = run_bass_kernel_spmd(
        nc, in_maps, core_ids=list(range(NCORES)), trace=trace
    )
    scores = np.empty(h.shape[0], dtype=np.float32)
    for k in range(NCORES):
        o = res.results[k]["out"]
        slots, positions, descale = slot_maps[k]
        vals = o[slots % 128, slots // 128]
        if descale is not None:
            vals = (vals.astype(np.float64) * descale).astype(np.float32)
        scores[positions] = vals
    return scores, res


def kernel(h, r, t, ent_weight, rel_weight):
    scores, _ = _run(h, r, t, ent_weight, rel_weight, trace=False)
    return scores



# revision 4
# speedup vs baseline: 1.0821x; 1.0821x over previous
"""Bilinear LTN scoring kernel for Trainium2 (8 NeuronCores).

scores[i] = ent[h[i]]^T @ W[r[i]] @ ent[t[i]],  B=4096, DIM=256.

Strategy: items grouped by relation (<=32 per group); groups sharded
across the 8 cores so each relation matrix streams from HBM once
system-wide. Per core ~63 groups; the kernel is HBM-bound on the W
stream and everything else hides under it.

Layout (per core; the *structure* is identical across cores, only the
data differs — SPMD shares one program):
  - group g owns grid slots [32g, 32g+32); grid block m = groups
    [4m, 4m+4) = one [128, 256] PSUM tile (4 stationary strips of 32).
  - groups are packed into "windows" of <=12 groups / <=128 items.
    Window w owns grid blocks [3w, 3w+3) (the last window may own
    fewer). Each window's items live compactly in 128 rows.
  - h/t embedding rows are gathered ON THE HOST into hct
    [128, NWIN*512] fp16 (cols [w*512, w*512+256) = h rows of window
    w, next 256 = t rows). This kills the SWDGE indirect-DMA chain
    (12 serial Q7 descriptor generations ~2.9us each) that used to
    pace the whole kernel; the HW now streams everything via HWDGE.
  - per (window, k-half): ONE PE matmul transposes+expands compact H
    into grid layout: htp[a, s] = sum_c hc[c, 128k+a] * P[c, s]
    (P one-hot, [128, <=384], streamed from DRAM pre-transposed).
  - per block: one PE matmul expands compact t into grid rows:
    texp[s, :] = sum_c P[c, s] tc[c, :] (PSUM fp32), then an ACT
    copy with per-partition scale (dscale, fp32 [128, NBLK]) moves it
    to SBUF — the scale carries the fp8 dequant factor 1/s_g (1.0 in
    fp16 mode).
  - W host-packed [128, G*512]; each window's chunk is split in two
    half-DMAs, one per HWDGE ring (sync + scalar), so both rings
    carry equal bytes and chunks complete in consumption order at
    aggregate bandwidth.
  - per block: 8 matmuls (2 contract halves x 4 strips of 32,
    tile_position-packed) accumulate h^T W in PSUM; then ONE fused
    DVE tensor_tensor_reduce (mult + row-sum) -> 128 scores.
  - 48 PE warm-up matmuls keep the HAM clock gate up until data
    arrives (cold/mid p-state makes every matmul ~2.3x slower).
"""

import sys

for _p in ("/opt/trn_rl_repo",):
    if _p not in sys.path:
        sys.path.insert(0, _p)

import ml_dtypes
import numpy as np

import concourse.bass as bass
import concourse.mybir as mybir
import concourse.tile as tile
from concourse.bass_utils import run_bass_kernel_spmd
from concourse.vector_clock import ScopedClock

DIM = 256
N_ENT = 100000
N_REL = 500
NCORES = 8
C = 32                 # grid slots per group (matmul stationary width)
WGROUPS = 12           # groups per window (3 grid blocks)

F32 = mybir.dt.float32
FP16 = mybir.dt.float16
I32 = mybir.dt.int32

MODE = "fp16"  # kept for test.py compatibility

# W stream dtype: False = fp16; True = float8_e3m4 with per-group scale
# (dequantized on-device via the dscale'd ACT copy of texp).
W8 = False
W8_TARGET = 14.0  # scale absmax of each W to this (e3m4 max is 15.5)

WARMUP = 48

_MAX_WAITS = 1


def _install_walrus_fixes():
    """This container's walrus accepts only one sync wait per instruction;
    split extra waits onto preceding same-engine NOPs."""
    if getattr(tile.TileContext, "_drain_fix_installed", False):
        return

    def _split_multi_waits(nc):
        cur_bb = nc.cur_bb.bb
        for f in nc.m.functions:
            for blk in f.blocks:
                bb = blk if hasattr(blk, "instructions") else blk.bb
                i = 0
                while i < len(bb.instructions):
                    inst = bb.instructions[i]
                    si = getattr(inst, "sync_info", None)
                    waits = list(si.on_wait or []) if si is not None else []
                    if len(waits) > _MAX_WAITS:
                        si.on_wait = waits[-_MAX_WAITS:]
                        extra = waits[: -_MAX_WAITS]
                        nops = []
                        for w0 in range(0, len(extra), _MAX_WAITS):
                            nop_inst = nc.engines[inst.engine].nop(
                                nofuse=True, hint="wait_split"
                            )
                            nop_inst.ins.sync_info = mybir.SyncInfo(
                                on_wait=extra[w0 : w0 + _MAX_WAITS],
                                on_update=[],
                            )
                            nops.append(nop_inst.ins)
                        for n in nops:
                            cur_bb.instructions.remove(n)
                        for j, n in enumerate(nops):
                            bb.instructions.insert(i + j, n)
                        i += len(nops)
                    i += 1

    def _drain_and_barrier(self, tick_clock, wait_clock):
        drain_inst = self.nc.sync.drain()
        wait_clock.add_sem_waits(
            drain_inst.ins, ScopedClock({None: tick_clock.global_clock})
        )
        self.nc.all_engine_barrier()
        assert self.sems is not None
        popped = self.nc._tile_sem_poison_stack.pop()
        assert popped is self._sem_poison
        self.nc.clear_and_free_semaphores(list(self.sems.allocated().values()))
        self.nc.all_engine_barrier()
        _split_multi_waits(self.nc)

    tile.TileContext._drain_and_barrier = _drain_and_barrier
    tile.TileContext._drain_fix_installed = True


def _wstride(comp):
    # pmat row stride = widest window's grid slots (384 for uniform windows)
    return max(comp) * 128


def _wcomp(NBLK, first=3):
    """Window block-composition."""
    comp = []
    left = NBLK
    while left > 0:
        b = min(first if not comp else 3, left)
        comp.append(b)
        left -= b
    return comp


def _build_bass(G, first=3, w8=None):
    _install_walrus_fixes()
    if w8 is None:
        w8 = W8
    WDT = mybir.dt.float8e3 if w8 else FP16
    PDT = mybir.dt.float8e3 if w8 else FP16
    NBLK = (G + 3) // 4
    comp = _wcomp(NBLK, first)
    NWIN = len(comp)
    bbase = [0]
    for b in comp:
        bbase.append(bbase[-1] + b)
    blk2win = [None] * NBLK
    for w in range(NWIN):
        for j in range(bbase[w], bbase[w + 1]):
            blk2win[j] = w
    WSTR = _wstride(comp)

    nc = bass.Bass()
    wpk = nc.declare_dram_parameter("wpk", [128, G * 512], WDT, isOutput=False)
    hct = nc.declare_dram_parameter("hct", [128, NWIN * 512], FP16, isOutput=False)
    pmat = nc.declare_dram_parameter("pmat", [128, NWIN * WSTR], PDT, isOutput=False)
    dscale = nc.declare_dram_parameter("dscale", [128, NBLK], F32, isOutput=False)
    out = nc.declare_dram_parameter("out", [128, NBLK], F32, isOutput=True)

    with tile.TileContext(nc) as tc:
        with (
            tc.tile_pool(name="const", bufs=1) as const_pool,
            tc.tile_pool(name="w", bufs=1) as w_pool,
            tc.tile_pool(name="sc", bufs=3) as sc_pool,
            tc.tile_pool(name="te", bufs=4) as te_sb_pool,
            tc.tile_pool(name="upsum", bufs=3, space="PSUM") as u_pool,
            tc.tile_pool(name="tepsum", bufs=3, space="PSUM") as te_pool,
            tc.tile_pool(name="trpsum", bufs=2, space="PSUM") as tr_pool,
        ):
            # ---- PE warm-up: dense dummy matmuls, no data deps
            dummy = const_pool.tile([128, DIM], FP16, tag="dummy")
            nc.vector.memset(dummy[:], 0.0)
            dps = u_pool.tile([128, DIM], F32, space="PSUM", tag="ups", name="dps")
            for wu in range(WARMUP):
                nc.tensor.matmul(
                    out=dps[0:32, :],
                    lhsT=dummy[:, 0:32],
                    rhs=dummy[:],
                    start=True,
                    stop=True,
                    tile_position=(0, 0),
                )

            # dscale tiny + first on the scalar ring
            ds_t = const_pool.tile([128, NBLK], F32, tag="ds")
            nc.scalar.dma_start(out=ds_t[:], in_=dscale[:])
            # pay ACT's one-time activation-table load before the queue
            # gets busy (it otherwise lands on the critical tail)
            actwarm = const_pool.tile([128, 1], F32, tag="actwarm")
            nc.vector.memset(actwarm[:], 0.0)
            nc.scalar.copy(actwarm[:], actwarm[:])

            out_sb = const_pool.tile([128, NBLK], F32, tag="outsb")
            nc.vector.memset(out_sb[:], 0.0)

            # ---- per-window streams: hct + pmat slices + split W chunks.
            # sync ring: [hct_w, wA_w]*; scalar ring: [dscale, pt_w, wB_w]*
            # Both rings carry ~equal bytes so window chunks complete in
            # consumption order at aggregate HBM bandwidth.
            hcts = []
            pts = []
            wts = {}  # block m -> (tile, col offset)
            for w in range(NWIN):
                b0 = bbase[w]
                nb = comp[w]
                ws = nb * 128
                hw_t = const_pool.tile([128, 512], FP16, tag=f"hct{w}", name=f"hct{w}")
                nc.sync.dma_start(out=hw_t[:], in_=hct[:, w * 512 : (w + 1) * 512])
                hcts.append(hw_t)
                pt_t = const_pool.tile([128, ws], PDT, tag=f"pt{w}", name=f"pt{w}")
                nc.scalar.dma_start(out=pt_t[:], in_=pmat[:, w * WSTR : w * WSTR + ws])
                pts.append(pt_t)
                ncols = min(G * 512, (b0 + nb) * 2048) - b0 * 2048
                half = (ncols // 2 + 511) // 512 * 512  # 512-col aligned split
                wt = w_pool.tile(
                    [128, nb * 2048], WDT, tag=f"wt{w}", name=f"wt{w}"
                )
                for m in range(b0, b0 + nb):
                    wts[m] = (wt, (m - b0) * 2048)
                nc.sync.dma_start(
                    out=wt[:, :half],
                    in_=wpk[:, b0 * 2048 : b0 * 2048 + half],
                )
                nc.scalar.dma_start(
                    out=wt[:, half:ncols],
                    in_=wpk[:, b0 * 2048 + half : b0 * 2048 + ncols],
                )

            # ---- per-window: H transpose+expand, then that window's blocks
            htps = {}

            def _emit_htrp(w):
                # htp_w_k[a, s] = sum_c hc[c, 128k + a] * P_w[c, s]
                ws = comp[w] * 128
                for k in range(2):
                    trp = tr_pool.tile(
                        [128, ws], F32, space="PSUM", tag="trp",
                        name=f"trp{w}_{k}",
                    )
                    nc.tensor.matmul(
                        out=trp[:],
                        lhsT=hcts[w][:, k * 128 : (k + 1) * 128],
                        rhs=pts[w][:],
                        start=True,
                        stop=True,
                    )
                    htp = const_pool.tile(
                        [128, ws], FP16, tag=f"htp{w}_{k}", name=f"htp{w}_{k}"
                    )
                    if k == 0:
                        nc.vector.tensor_copy(htp[:], trp[:])
                    else:
                        nc.scalar.copy(htp[:], trp[:])
                    htps[(w, k)] = htp

            for m in range(NBLK):
                nstrip = min(G, 4 * m + 4) - 4 * m
                w = blk2win[m]
                jloc = m - bbase[w]
                npart = 32 * nstrip
                if (w, 0) not in htps:
                    _emit_htrp(w)

                texp = te_pool.tile(
                    [128, DIM], F32, space="PSUM", tag="texp", name=f"texp{m}"
                )
                nc.tensor.matmul(
                    out=texp[0:npart, :],
                    lhsT=pts[w][:, jloc * 128 : jloc * 128 + npart],
                    rhs=hcts[w][:, 256:512],
                    start=True,
                    stop=True,
                )
                texp_sb = te_sb_pool.tile(
                    [128, DIM], F32, tag="tesb", name=f"tesb{m}"
                )
                # dequant fold: texp_sb = texp * (1/s_g per slot)
                nc.scalar.mul(
                    texp_sb[0:npart, :], texp[0:npart, :],
                    ds_t[0:npart, m : m + 1],
                )

                ups = u_pool.tile(
                    [128, DIM], F32, space="PSUM", tag="ups", name=f"ups{m}"
                )
                wtile, wcol = wts[m]
                for k in range(2):
                    for d in range(nstrip):
                        nc.tensor.matmul(
                            out=ups[32 * d : 32 * d + 32, :],
                            lhsT=htps[(w, k)][
                                :, jloc * 128 + 32 * d : jloc * 128 + 32 * d + 32
                            ],
                            rhs=wtile[
                                :,
                                wcol
                                + (d * 2 + k) * DIM : wcol
                                + (d * 2 + k + 1) * DIM,
                            ],
                            start=(k == 0),
                            stop=(k == 1),
                            tile_position=(0, 32 * d),
                            skip_group_check=True,
                        )

                sc = sc_pool.tile([128, DIM], F32, tag="sc", name=f"sc{m}")
                nc.vector.tensor_tensor(
                    out=sc[0:npart, :],
                    in0=ups[0:npart, :],
                    in1=texp_sb[0:npart, :],
                    op=mybir.AluOpType.mult,
                )
                nc.vector.tensor_reduce(
                    out=out_sb[0:npart, m : m + 1],
                    in_=sc[0:npart, :],
                    axis=mybir.AxisListType.X,
                    op=mybir.AluOpType.add,
                )
                if m == 11 and NBLK > 12:
                    # early partial writeback: the final DMA then waits only
                    # on the last blocks' reduces
                    nc.sync.dma_start(
                        out=out[:, 0:12], in_=out_sb[:, 0:12]
                    )

            if NBLK > 12:
                nc.sync.dma_start(out=out[:, 12:], in_=out_sb[:, 12:])
            else:
                nc.sync.dma_start(out=out[:], in_=out_sb[:])

    return nc


_NC_CACHE = {}


def _get_nc(G, first):
    key = (G, first, W8)
    if key not in _NC_CACHE:
        _NC_CACHE[key] = _build_bass(G, first)
    return _NC_CACHE[key]


def _pack(h, r, t, ent_weight, rel_weight, first=3):
    """Group items by relation (chunks <=C), balance chunks across cores,
    pack each core's chunks into windows; host-gather the h/t embedding
    rows into per-window compact fp16 tiles."""
    order = np.argsort(r, kind="stable")
    rs = r[order]
    starts = np.flatnonzero(np.r_[True, rs[1:] != rs[:-1]])
    ends = np.r_[starts[1:], len(rs)]
    chunks = []  # (rel_id, item_positions)
    for s0, e0 in zip(starts, ends):
        rid = int(rs[s0])
        for c0 in range(s0, e0, C):
            chunks.append((rid, order[c0 : min(c0 + C, e0)]))
    chunks.sort(key=lambda x: -len(x[1]))

    per_core = [[] for _ in range(NCORES)]
    counts = [0] * NCORES
    items = [0] * NCORES
    for ch in chunks:
        k = min(range(NCORES), key=lambda q: (counts[q], items[q]))
        per_core[k].append(ch)
        counts[k] += 1
        items[k] += len(ch[1])

    G = max(counts)
    NBLK = (G + 3) // 4
    comp = _wcomp(NBLK, first)
    NWIN = len(comp)
    bbase = [0]
    for b in comp:
        bbase.append(bbase[-1] + b)
    # window wi holds exactly quota[wi] group slots (last takes remainder)
    quota = [min(4 * comp[wi], G - 4 * bbase[wi]) for wi in range(NWIN)]
    WSTR = _wstride(comp)

    wdt = ml_dtypes.float8_e3m4 if W8 else np.float16
    in_maps = []
    slot_maps = []
    for k in range(NCORES):
        # assign chunks to windows (greedy on item counts, sorted desc;
        # every window must fill to its quota and stay <=128 items)
        wins = [[] for _ in range(NWIN)]
        witems = [0] * NWIN
        for ch in per_core[k]:
            n = len(ch[1])
            cand = [
                wi
                for wi in range(NWIN)
                if len(wins[wi]) < quota[wi] and witems[wi] + n <= 128
            ]
            if not cand:  # fall through; the overflow assert fires below
                cand = [
                    wi for wi in range(NWIN) if len(wins[wi]) < quota[wi]
                ]
            wi = min(cand, key=lambda q: witems[q])
            wins[wi].append(ch)
            witems[wi] += n
        assert max(witems) <= 128, f"window overflow: {witems}"

        wpk = np.zeros((128, G * 512), dtype=wdt)
        hct = np.zeros((128, NWIN * 512), dtype=np.float16)
        pmat = np.zeros((128, NWIN * WSTR), dtype=np.float16)
        dscale = np.ones((128, NBLK), dtype=np.float32)
        slots = []
        positions = []
        for wi in range(NWIN):
            cpos = 0
            for gl, (rid, pos) in enumerate(wins[wi]):
                g = 4 * bbase[wi] + gl
                w3 = (
                    rel_weight[rid]
                    .reshape(2, 128, DIM)
                    .transpose(1, 0, 2)
                    .reshape(128, 512)
                )
                s = g * C + np.arange(len(pos))
                if W8:
                    sg = W8_TARGET / max(np.abs(w3).max(), 1e-30)
                    wpk[:, g * 512 : (g + 1) * 512] = (w3 * sg).astype(wdt)
                    dscale[s % 128, s // 128] = 1.0 / sg
                else:
                    wpk[:, g * 512 : (g + 1) * 512] = w3.astype(wdt)
                cslot = cpos + np.arange(len(pos))
                cpos += len(pos)
                hct[cslot, wi * 512 : wi * 512 + 256] = ent_weight[
                    h[pos]
                ].astype(np.float16)
                hct[cslot, wi * 512 + 256 : wi * 512 + 512] = ent_weight[
                    t[pos]
                ].astype(np.float16)
                # window wi owns grid slots from 128*bbase[wi]
                local_s = s - 128 * bbase[wi]
                pmat[cslot, wi * WSTR + local_s] = 1.0
                slots.append(s)
                positions.append(pos)
        slots = np.concatenate(slots) if slots else np.zeros(0, np.int64)
        positions = (
            np.concatenate(positions) if positions else np.zeros(0, np.int64)
        )
        slot_maps.append((slots, positions))
        in_maps.append(
            {
                "wpk": wpk,
                "hct": hct,
                "pmat": pmat.astype(
                    ml_dtypes.float8_e3m4 if W8 else np.float16
                ),
                "dscale": dscale,
            }
        )
    return in_maps, slot_maps, G


def _run(h, r, t, ent_weight, rel_weight, trace=False, mode=None):
    h = np.asarray(h).astype(np.int64)
    r = np.asarray(r).astype(np.int64)
    t = np.asarray(t).astype(np.int64)
    ent_weight = np.ascontiguousarray(np.asarray(ent_weight, dtype=np.float32))
    rel_weight = np.ascontiguousarray(np.asarray(rel_weight, dtype=np.float32))
    assert ent_weight.shape == (N_ENT, DIM)
    assert rel_weight.shape == (N_REL, DIM * DIM)

    first = 3
    in_maps, slot_maps, G = _pack(h, r, t, ent_weight, rel_weight, first=first)
    nc = _get_nc(G, first)
    res = run_bass_kernel_spmd(
        nc, in_maps, core_ids=list(range(NCORES)), trace=trace
    )
    scores = np.empty(h.shape[0], dtype=np.float32)
    for k in range(NCORES):
        o = res.results[k]["out"]
        slots, positions = slot_maps[k]
        scores[positions] = o[slots % 128, slots // 128]
    return scores, res


def kernel(h, r, t, ent_weight, rel_weight):
    scores, _ = _run(h, r, t, ent_weight, rel_weight, trace=False)
    return scores


# revision 15
# speedup vs baseline: 1.2055x; 1.1140x over previous
"""Bilinear LTN scoring kernel for Trainium2 (8 NeuronCores).

scores[i] = ent[h[i]]^T @ W[r[i]] @ ent[t[i]],  B=4096, DIM=256.

Strategy: items grouped by relation (<=32 per group); groups sharded
across the 8 cores so each relation matrix streams from HBM once
system-wide (fp8 e3m4, per-group absmax scale). Per core ~63 groups;
the kernel pipelines a ~5.9MB HWDGE stream against dense PE compute.

Layout (per core; the *structure* is identical across cores, only the
data differs — SPMD shares one program):
  - group g owns grid slots [32g, 32g+32); grid block m = groups
    [4m, 4m+4) = one [128, 256] PSUM tile (4 stationary strips of 32).
  - groups are packed into "windows" of <=12 groups / <=128 items
    (window w owns grid blocks [3w, 3w+3); last may own fewer).
  - ALL gather/transpose work is done on the host at pack time:
      hT  [128, G*64]  fp16: hT[a, g*64+k*32+j] = h_emb[j of g][128k+a]
          — the W-matmul stationary operand, directly in lhsT layout.
      tcc [128, NWIN*256] fp16: compact t rows per window.
      pmat[128, NWIN*WSTR] one-hot: expands compact t rows to grid
          slots via one PE matmul per block (texp).
      wpk [128, G*512] fp8/fp16: W row-pairs (contract halves
          interleaved per group).
      dscale [128, NBLK] f32: per-slot 1/s_g fp8 dequant factors.
  - per block: texp matmul -> PSUM; ACT copy with per-partition
    scale=dscale (the fp8 dequant, free) -> SBUF; 8 PE matmuls
    (2 contract halves x 4 strips of 32, tile_position-packed)
    accumulate h^T W in PSUM; DVE mult (ups x texp_sb); GPSIMD
    row-sum -> 128 scores. Consumers are spread over ACT/DVE/Pool so
    no single engine's serial chain back-pressures the PE: PSUM-pool
    recycling stalls otherwise drop the PE to its slow p-state
    (cold/mid makes every matmul ~2.3-3.7x slower).
  - every W window chunk is split into two half-DMAs, one per HWDGE
    ring (sync + scalar), so both rings carry equal bytes and chunks
    complete in consumption order at aggregate HBM bandwidth.
  - PE warm-up matmuls bridge from t=0 until the first W chunk lands
    so the PE hits the first real matmul at full clock.
"""

import sys

for _p in ("/opt/trn_rl_repo",):
    if _p not in sys.path:
        sys.path.insert(0, _p)

import ml_dtypes
import numpy as np

import concourse.bass as bass
import concourse.mybir as mybir
import concourse.tile as tile
from concourse.bass_utils import run_bass_kernel_spmd
from concourse.vector_clock import ScopedClock

DIM = 256
N_ENT = 100000
N_REL = 500
NCORES = 8
C = 32                 # grid slots per group (matmul stationary width)

F32 = mybir.dt.float32
FP16 = mybir.dt.float16

MODE = "fp16"  # kept for test.py compatibility

# W stream dtype: False = fp16; True = float8_e3m4 with per-group scale
# (dequantized on-device via the dscale'd ACT copy of texp).
W8 = True
W8_TARGET = 14.0  # scale absmax of each W to this (e3m4 max is 15.5)

WARMUP = 68

_MAX_WAITS = 1


def _install_walrus_fixes():
    """This container's walrus accepts only one sync wait per instruction;
    split extra waits onto preceding same-engine NOPs."""
    if getattr(tile.TileContext, "_drain_fix_installed", False):
        return

    def _split_multi_waits(nc):
        cur_bb = nc.cur_bb.bb
        for f in nc.m.functions:
            for blk in f.blocks:
                bb = blk if hasattr(blk, "instructions") else blk.bb
                i = 0
                while i < len(bb.instructions):
                    inst = bb.instructions[i]
                    si = getattr(inst, "sync_info", None)
                    waits = list(si.on_wait or []) if si is not None else []
                    if len(waits) > _MAX_WAITS:
                        si.on_wait = waits[-_MAX_WAITS:]
                        extra = waits[: -_MAX_WAITS]
                        nops = []
                        for w0 in range(0, len(extra), _MAX_WAITS):
                            nop_inst = nc.engines[inst.engine].nop(
                                nofuse=True, hint="wait_split"
                            )
                            nop_inst.ins.sync_info = mybir.SyncInfo(
                                on_wait=extra[w0 : w0 + _MAX_WAITS],
                                on_update=[],
                            )
                            nops.append(nop_inst.ins)
                        for n in nops:
                            cur_bb.instructions.remove(n)
                        for j, n in enumerate(nops):
                            bb.instructions.insert(i + j, n)
                        i += len(nops)
                    i += 1

    def _drain_and_barrier(self, tick_clock, wait_clock):
        drain_inst = self.nc.sync.drain()
        wait_clock.add_sem_waits(
            drain_inst.ins, ScopedClock({None: tick_clock.global_clock})
        )
        self.nc.all_engine_barrier()
        assert self.sems is not None
        popped = self.nc._tile_sem_poison_stack.pop()
        assert popped is self._sem_poison
        self.nc.clear_and_free_semaphores(list(self.sems.allocated().values()))
        self.nc.all_engine_barrier()
        _split_multi_waits(self.nc)

    tile.TileContext._drain_and_barrier = _drain_and_barrier
    tile.TileContext._drain_fix_installed = True


def _wstride(comp):
    # pmat row stride = widest window's grid slots (384 for uniform windows)
    return max(comp) * 128


def _wcomp(NBLK, first=3):
    """Window block-composition."""
    comp = []
    left = NBLK
    while left > 0:
        b = min(first if not comp else 3, left)
        comp.append(b)
        left -= b
    return comp


def _build_bass(G, first=3, w8=None):
    _install_walrus_fixes()
    if w8 is None:
        w8 = W8
    WDT = mybir.dt.float8e3 if w8 else FP16
    PDT = mybir.dt.float8e3 if w8 else FP16
    NBLK = (G + 3) // 4
    comp = _wcomp(NBLK, first)
    NWIN = len(comp)
    bbase = [0]
    for b in comp:
        bbase.append(bbase[-1] + b)
    blk2win = [None] * NBLK
    for w in range(NWIN):
        for j in range(bbase[w], bbase[w + 1]):
            blk2win[j] = w
    WSTR = _wstride(comp)
    # window w covers groups [gw0[w], gw0[w+1])
    gw0 = [min(4 * bbase[w], G) for w in range(NWIN)] + [G]

    nc = bass.Bass()
    wpk = nc.declare_dram_parameter("wpk", [128, G * 512], WDT, isOutput=False)
    hT = nc.declare_dram_parameter("hT", [128, G * 64], FP16, isOutput=False)
    tcc = nc.declare_dram_parameter("tcc", [128, NWIN * 256], FP16, isOutput=False)
    pmat = nc.declare_dram_parameter("pmat", [128, NWIN * WSTR], PDT, isOutput=False)
    dscale = nc.declare_dram_parameter("dscale", [128, NBLK], F32, isOutput=False)
    out = nc.declare_dram_parameter("out", [128, NBLK], F32, isOutput=True)

    with tile.TileContext(nc) as tc:
        with (
            tc.tile_pool(name="const", bufs=1) as const_pool,
            tc.tile_pool(name="w", bufs=1) as w_pool,
            tc.tile_pool(name="sc", bufs=4) as sc_pool,
            tc.tile_pool(name="te", bufs=4) as te_sb_pool,
            tc.tile_pool(name="upsum", bufs=4, space="PSUM") as u_pool,
            tc.tile_pool(name="tepsum", bufs=4, space="PSUM") as te_pool,
        ):
            # ---- PE warm-up: dense dummy matmuls, no data deps
            dummy = const_pool.tile([128, DIM], FP16, tag="dummy")
            nc.vector.memset(dummy[:], 0.0)
            dps = u_pool.tile([128, DIM], F32, space="PSUM", tag="ups", name="dps")
            for wu in range(WARMUP):
                nc.tensor.matmul(
                    out=dps[0:32, :],
                    lhsT=dummy[:, 0:32],
                    rhs=dummy[:],
                    start=True,
                    stop=True,
                    tile_position=(0, 0),
                )

            # dscale tiny + first on the scalar ring
            ds_t = const_pool.tile([128, NBLK], F32, tag="ds")
            nc.scalar.dma_start(out=ds_t[:], in_=dscale[:])
            # pay ACT's one-time activation-table load before the queue
            # gets busy (it otherwise lands on the critical tail)
            actwarm = const_pool.tile([128, 1], F32, tag="actwarm")
            nc.vector.memset(actwarm[:], 0.0)
            nc.scalar.copy(actwarm[:], actwarm[:])

            out_sb = const_pool.tile([128, NBLK], F32, tag="outsb")
            nc.vector.memset(out_sb[:], 0.0)

            # ---- per-window streams: hT/tcc/pmat slices + split W chunks.
            # sync ring: [hT_w, wA_w]*; scalar ring: [ds, tcc_w, pt_w, wB_w]*
            hts = []
            tcs = []
            pts = []
            wts = {}  # block m -> (tile, col offset)
            for w in range(NWIN):
                b0 = bbase[w]
                nb = comp[w]
                ws = nb * 128
                ng = gw0[w + 1] - gw0[w]
                ht_t = const_pool.tile(
                    [128, ng * 64], FP16, tag=f"ht{w}", name=f"ht{w}"
                )
                nc.sync.dma_start(
                    out=ht_t[:], in_=hT[:, gw0[w] * 64 : gw0[w + 1] * 64]
                )
                hts.append(ht_t)
                tc_t = const_pool.tile([128, 256], FP16, tag=f"tc{w}", name=f"tc{w}")
                nc.scalar.dma_start(
                    out=tc_t[:], in_=tcc[:, w * 256 : (w + 1) * 256]
                )
                tcs.append(tc_t)
                pt_t = const_pool.tile([128, ws], PDT, tag=f"pt{w}", name=f"pt{w}")
                nc.scalar.dma_start(out=pt_t[:], in_=pmat[:, w * WSTR : w * WSTR + ws])
                pts.append(pt_t)
                ncols = min(G * 512, (b0 + nb) * 2048) - b0 * 2048
                half = (ncols // 2 + 511) // 512 * 512  # 512-col aligned split
                wt = w_pool.tile(
                    [128, nb * 2048], WDT, tag=f"wt{w}", name=f"wt{w}"
                )
                for m in range(b0, b0 + nb):
                    wts[m] = (wt, (m - b0) * 2048)
                nc.sync.dma_start(
                    out=wt[:, :half],
                    in_=wpk[:, b0 * 2048 : b0 * 2048 + half],
                )
                nc.scalar.dma_start(
                    out=wt[:, half:ncols],
                    in_=wpk[:, b0 * 2048 + half : b0 * 2048 + ncols],
                )

            for m in range(NBLK):
                nstrip = min(G, 4 * m + 4) - 4 * m
                w = blk2win[m]
                jloc = m - bbase[w]
                npart = 32 * nstrip

                texp = te_pool.tile(
                    [128, DIM], F32, space="PSUM", tag="texp", name=f"texp{m}"
                )
                nc.tensor.matmul(
                    out=texp[0:npart, :],
                    lhsT=pts[w][:, jloc * 128 : jloc * 128 + npart],
                    rhs=tcs[w][:],
                    start=True,
                    stop=True,
                )
                texp_sb = te_sb_pool.tile(
                    [128, DIM], F32, tag="tesb", name=f"tesb{m}"
                )
                # fp8 dequant folded in: texp_sb = texp * (1/s_g per slot)
                nc.scalar.mul(
                    texp_sb[0:npart, :], texp[0:npart, :],
                    ds_t[0:npart, m : m + 1],
                )

                ups = u_pool.tile(
                    [128, DIM], F32, space="PSUM", tag="ups", name=f"ups{m}"
                )
                wtile, wcol = wts[m]
                hbase = (4 * m - gw0[w]) * 64
                for k in range(2):
                    for d in range(nstrip):
                        nc.tensor.matmul(
                            out=ups[32 * d : 32 * d + 32, :],
                            lhsT=hts[w][
                                :,
                                hbase + d * 64 + k * 32 : hbase + d * 64 + k * 32 + 32,
                            ],
                            rhs=wtile[
                                :,
                                wcol
                                + (d * 2 + k) * DIM : wcol
                                + (d * 2 + k + 1) * DIM,
                            ],
                            start=(k == 0),
                            stop=(k == 1),
                            tile_position=(0, 32 * d),
                            skip_group_check=True,
                        )

                sc = sc_pool.tile([128, DIM], F32, tag="sc", name=f"sc{m}")
                nc.vector.tensor_tensor(
                    out=sc[0:npart, :],
                    in0=ups[0:npart, :],
                    in1=texp_sb[0:npart, :],
                    op=mybir.AluOpType.mult,
                )
                nc.vector.tensor_reduce(
                    out=out_sb[0:npart, m : m + 1],
                    in_=sc[0:npart, :],
                    axis=mybir.AxisListType.X,
                    op=mybir.AluOpType.add,
                )
                if m == 11 and NBLK > 12:
                    # early partial writeback: the final DMA then waits only
                    # on the last blocks' reduces
                    nc.sync.dma_start(
                        out=out[:, 0:12], in_=out_sb[:, 0:12]
                    )

            if NBLK > 12:
                nc.sync.dma_start(out=out[:, 12:], in_=out_sb[:, 12:])
            else:
                nc.sync.dma_start(out=out[:], in_=out_sb[:])

    return nc


_NC_CACHE = {}


def _get_nc(G, first):
    key = (G, first, W8)
    if key not in _NC_CACHE:
        _NC_CACHE[key] = _build_bass(G, first)
    return _NC_CACHE[key]


def _pack(h, r, t, ent_weight, rel_weight, first=3):
    """Group items by relation (chunks <=C), balance chunks across cores,
    pack each core's chunks into windows; host-gather h into transposed
    lhsT layout (hT) and t into compact per-window rows (tcc)."""
    order = np.argsort(r, kind="stable")
    rs = r[order]
    starts = np.flatnonzero(np.r_[True, rs[1:] != rs[:-1]])
    ends = np.r_[starts[1:], len(rs)]
    chunks = []  # (rel_id, item_positions)
    for s0, e0 in zip(starts, ends):
        rid = int(rs[s0])
        for c0 in range(s0, e0, C):
            chunks.append((rid, order[c0 : min(c0 + C, e0)]))
    chunks.sort(key=lambda x: -len(x[1]))

    per_core = [[] for _ in range(NCORES)]
    counts = [0] * NCORES
    items = [0] * NCORES
    for ch in chunks:
        k = min(range(NCORES), key=lambda q: (counts[q], items[q]))
        per_core[k].append(ch)
        counts[k] += 1
        items[k] += len(ch[1])

    G = max(counts)
    NBLK = (G + 3) // 4
    comp = _wcomp(NBLK, first)
    NWIN = len(comp)
    bbase = [0]
    for b in comp:
        bbase.append(bbase[-1] + b)
    quota = [min(4 * comp[wi], G - 4 * bbase[wi]) for wi in range(NWIN)]
    WSTR = _wstride(comp)

    wdt = ml_dtypes.float8_e3m4 if W8 else np.float16
    in_maps = []
    slot_maps = []
    for k in range(NCORES):
        # assign chunks to windows (greedy on item counts, sorted desc;
        # every window must fill to its quota and stay <=128 items)
        wins = [[] for _ in range(NWIN)]
        witems = [0] * NWIN
        for ch in per_core[k]:
            n = len(ch[1])
            cand = [
                wi
                for wi in range(NWIN)
                if len(wins[wi]) < quota[wi] and witems[wi] + n <= 128
            ]
            if not cand:  # fall through; the overflow assert fires below
                cand = [
                    wi for wi in range(NWIN) if len(wins[wi]) < quota[wi]
                ]
            wi = min(cand, key=lambda q: witems[q])
            wins[wi].append(ch)
            witems[wi] += n
        assert max(witems) <= 128, f"window overflow: {witems}"

        wpk = np.zeros((128, G * 512), dtype=wdt)
        hT = np.zeros((128, G * 64), dtype=np.float16)
        tcc = np.zeros((128, NWIN * 256), dtype=np.float16)
        pmat = np.zeros((128, NWIN * WSTR), dtype=np.float16)
        dscale = np.ones((128, NBLK), dtype=np.float32)
        slots = []
        positions = []
        for wi in range(NWIN):
            cpos = 0
            for gl, (rid, pos) in enumerate(wins[wi]):
                g = 4 * bbase[wi] + gl
                w3 = (
                    rel_weight[rid]
                    .reshape(2, 128, DIM)
                    .transpose(1, 0, 2)
                    .reshape(128, 512)
                )
                s = g * C + np.arange(len(pos))
                if W8:
                    sg = W8_TARGET / max(np.abs(w3).max(), 1e-30)
                    wpk[:, g * 512 : (g + 1) * 512] = (w3 * sg).astype(wdt)
                    dscale[s % 128, s // 128] = 1.0 / sg
                else:
                    wpk[:, g * 512 : (g + 1) * 512] = w3.astype(wdt)
                n = len(pos)
                he = ent_weight[h[pos]].astype(np.float16)  # [n, 256]
                hT[:, g * 64 : g * 64 + n] = he[:, 0:128].T
                hT[:, g * 64 + 32 : g * 64 + 32 + n] = he[:, 128:256].T
                cslot = cpos + np.arange(n)
                cpos += n
                tcc[cslot, wi * 256 : (wi + 1) * 256] = ent_weight[
                    t[pos]
                ].astype(np.float16)
                # window wi owns grid slots from 128*bbase[wi]
                local_s = s - 128 * bbase[wi]
                pmat[cslot, wi * WSTR + local_s] = 1.0
                slots.append(s)
                positions.append(pos)
        slots = np.concatenate(slots) if slots else np.zeros(0, np.int64)
        positions = (
            np.concatenate(positions) if positions else np.zeros(0, np.int64)
        )
        slot_maps.append((slots, positions))
        in_maps.append(
            {
                "wpk": wpk,
                "hT": hT,
                "tcc": tcc,
                "pmat": pmat.astype(
                    ml_dtypes.float8_e3m4 if W8 else np.float16
                ),
                "dscale": dscale,
            }
        )
    return in_maps, slot_maps, G


def _run(h, r, t, ent_weight, rel_weight, trace=False, mode=None):
    h = np.asarray(h).astype(np.int64)
    r = np.asarray(r).astype(np.int64)
    t = np.asarray(t).astype(np.int64)
    ent_weight = np.ascontiguousarray(np.asarray(ent_weight, dtype=np.float32))
    rel_weight = np.ascontiguousarray(np.asarray(rel_weight, dtype=np.float32))
    assert ent_weight.shape == (N_ENT, DIM)
    assert rel_weight.shape == (N_REL, DIM * DIM)

    first = 3
    in_maps, slot_maps, G = _pack(h, r, t, ent_weight, rel_weight, first=first)
    nc = _get_nc(G, first)
    res = run_bass_kernel_spmd(
        nc, in_maps, core_ids=list(range(NCORES)), trace=trace
    )
    scores = np.empty(h.shape[0], dtype=np.float32)
    for k in range(NCORES):
        o = res.results[k]["out"]
        slots, positions = slot_maps[k]
        scores[positions] = o[slots % 128, slots // 128]
    return scores, res


def kernel(h, r, t, ent_weight, rel_weight):
    scores, _ = _run(h, r, t, ent_weight, rel_weight, trace=False)
    return scores


# revision 19
# speedup vs baseline: 1.5986x; 1.3261x over previous
"""Bilinear LTN scoring kernel for Trainium2 (8 NeuronCores).

scores[i] = ent[h[i]]^T @ W[r[i]] @ ent[t[i]],  B=4096, DIM=256.

Strategy: items grouped by relation (<=32 per group); groups sharded
across the 8 cores so each relation matrix streams from HBM once
system-wide (fp8 e3m4, per-group absmax scale). Per core ~63 groups;
the kernel pipelines a ~5.9MB HWDGE stream against dense PE compute.

Layout (per core; the *structure* is identical across cores, only the
data differs — SPMD shares one program):
  - group g owns grid slots [32g, 32g+32); grid block m = groups
    [4m, 4m+4) = one [128, 256] PSUM tile (4 stationary strips of 32).
  - groups are packed into "windows" of <=12 groups / <=128 items
    (window w owns grid blocks [3w, 3w+3); last may own fewer).
  - ALL gather/transpose work is done on the host at pack time:
      hT  [128, G*64]  fp16: hT[a, g*64+k*32+j] = h_emb[j of g][128k+a]
          — the W-matmul stationary operand, directly in lhsT layout.
      tcc [128, NWIN*256] fp16: compact t rows per window.
      pmat[128, NWIN*WSTR] one-hot: expands compact t rows to grid
          slots via one PE matmul per block (texp).
      wpk [128, G*512] fp8/fp16: W row-pairs (contract halves
          interleaved per group).
      dscale [128, NBLK] f32: per-slot 1/s_g fp8 dequant factors.
  - per block: texp matmul -> PSUM; ACT copy with per-partition
    scale=dscale (the fp8 dequant, free) -> SBUF; 8 PE matmuls
    (2 contract halves x 4 strips of 32, tile_position-packed)
    accumulate h^T W in PSUM; DVE mult (ups x texp_sb); GPSIMD
    row-sum -> 128 scores. Consumers are spread over ACT/DVE/Pool so
    no single engine's serial chain back-pressures the PE: PSUM-pool
    recycling stalls otherwise drop the PE to its slow p-state
    (cold/mid makes every matmul ~2.3-3.7x slower).
  - every W window chunk is split into two half-DMAs, one per HWDGE
    ring (sync + scalar), so both rings carry equal bytes and chunks
    complete in consumption order at aggregate HBM bandwidth.
  - PE warm-up matmuls bridge from t=0 until the first W chunk lands
    so the PE hits the first real matmul at full clock.
"""

import sys

for _p in ("/opt/trn_rl_repo",):
    if _p not in sys.path:
        sys.path.insert(0, _p)

import ml_dtypes
import numpy as np

import concourse.bass as bass
import concourse.mybir as mybir
import concourse.tile as tile
from concourse.bass_utils import run_bass_kernel_spmd
from concourse.vector_clock import ScopedClock

DIM = 256
N_ENT = 100000
N_REL = 500
NCORES = 8
C = 32                 # grid slots per group (matmul stationary width)

F32 = mybir.dt.float32
FP16 = mybir.dt.float16

MODE = "fp16"  # kept for test.py compatibility

# W stream dtype: False = fp16; True = float8_e3m4 with per-group scale
# (dequantized on-device via the dscale'd ACT copy of texp).
W8 = True
W8_TARGET = 14.0  # scale absmax of each W to this (e3m4 max is 15.5)

WARMUP = 20

_MAX_WAITS = 1


def _install_walrus_fixes():
    """This container's walrus accepts only one sync wait per instruction;
    split extra waits onto preceding same-engine NOPs."""
    if getattr(tile.TileContext, "_drain_fix_installed", False):
        return

    def _split_multi_waits(nc):
        cur_bb = nc.cur_bb.bb
        for f in nc.m.functions:
            for blk in f.blocks:
                bb = blk if hasattr(blk, "instructions") else blk.bb
                i = 0
                while i < len(bb.instructions):
                    inst = bb.instructions[i]
                    si = getattr(inst, "sync_info", None)
                    waits = list(si.on_wait or []) if si is not None else []
                    if len(waits) > _MAX_WAITS:
                        si.on_wait = waits[-_MAX_WAITS:]
                        extra = waits[: -_MAX_WAITS]
                        nops = []
                        for w0 in range(0, len(extra), _MAX_WAITS):
                            nop_inst = nc.engines[inst.engine].nop(
                                nofuse=True, hint="wait_split"
                            )
                            nop_inst.ins.sync_info = mybir.SyncInfo(
                                on_wait=extra[w0 : w0 + _MAX_WAITS],
                                on_update=[],
                            )
                            nops.append(nop_inst.ins)
                        for n in nops:
                            cur_bb.instructions.remove(n)
                        for j, n in enumerate(nops):
                            bb.instructions.insert(i + j, n)
                        i += len(nops)
                    i += 1

    def _drain_and_barrier(self, tick_clock, wait_clock):
        drain_inst = self.nc.sync.drain()
        wait_clock.add_sem_waits(
            drain_inst.ins, ScopedClock({None: tick_clock.global_clock})
        )
        self.nc.all_engine_barrier()
        assert self.sems is not None
        popped = self.nc._tile_sem_poison_stack.pop()
        assert popped is self._sem_poison
        self.nc.clear_and_free_semaphores(list(self.sems.allocated().values()))
        self.nc.all_engine_barrier()
        _split_multi_waits(self.nc)

    tile.TileContext._drain_and_barrier = _drain_and_barrier
    tile.TileContext._drain_fix_installed = True


def _wstride(comp):
    # pmat row stride = widest window's grid slots (384 for uniform windows)
    return max(comp) * 128


def _wcomp(NBLK, first=3):
    """Window block-composition."""
    comp = []
    left = NBLK
    while left > 0:
        b = min(first if not comp else 3, left)
        comp.append(b)
        left -= b
    return comp


def _build_bass(G, first=3, w8=None):
    _install_walrus_fixes()
    if w8 is None:
        w8 = W8
    WDT = mybir.dt.float8e3 if w8 else FP16
    PDT = mybir.dt.float8e3 if w8 else FP16
    NBLK = (G + 3) // 4
    comp = _wcomp(NBLK, first)
    NWIN = len(comp)
    bbase = [0]
    for b in comp:
        bbase.append(bbase[-1] + b)
    blk2win = [None] * NBLK
    for w in range(NWIN):
        for j in range(bbase[w], bbase[w + 1]):
            blk2win[j] = w
    WSTR = _wstride(comp)
    # window w covers groups [gw0[w], gw0[w+1])
    gw0 = [min(4 * bbase[w], G) for w in range(NWIN)] + [G]

    nc = bass.Bass()
    wpk = nc.declare_dram_parameter("wpk", [128, G * 512], WDT, isOutput=False)
    hT = nc.declare_dram_parameter("hT", [128, G * 64], FP16, isOutput=False)
    tcc = nc.declare_dram_parameter("tcc", [128, NWIN * 256], FP16, isOutput=False)
    pmat = nc.declare_dram_parameter("pmat", [128, NWIN * WSTR], PDT, isOutput=False)
    dscale = nc.declare_dram_parameter("dscale", [128, NBLK], F32, isOutput=False)
    out = nc.declare_dram_parameter("out", [128, NBLK], F32, isOutput=True)

    with tile.TileContext(nc) as tc:
        with (
            tc.tile_pool(name="const", bufs=1) as const_pool,
            tc.tile_pool(name="w", bufs=1) as w_pool,
            tc.tile_pool(name="sc", bufs=4) as sc_pool,
            tc.tile_pool(name="te", bufs=4) as te_sb_pool,
            tc.tile_pool(name="upsum", bufs=4, space="PSUM") as u_pool,
            tc.tile_pool(name="tepsum", bufs=4, space="PSUM") as te_pool,
        ):
            # ---- PE warm-up: dense dummy matmuls, no data deps
            dummy = const_pool.tile([128, DIM], FP16, tag="dummy")
            nc.vector.memset(dummy[:], 0.0)
            dps = u_pool.tile([128, DIM], F32, space="PSUM", tag="ups", name="dps")
            for wu in range(WARMUP):
                nc.tensor.matmul(
                    out=dps[0:32, :],
                    lhsT=dummy[:, 0:32],
                    rhs=dummy[:],
                    start=True,
                    stop=True,
                    tile_position=(0, 0),
                )

            # dscale tiny + first on the scalar ring
            ds_t = const_pool.tile([128, NBLK], F32, tag="ds")
            nc.scalar.dma_start(out=ds_t[:], in_=dscale[:])
            # pay ACT's one-time activation-table load before the queue
            # gets busy (it otherwise lands on the critical tail)
            actwarm = const_pool.tile([128, 1], F32, tag="actwarm")
            nc.vector.memset(actwarm[:], 0.0)
            nc.scalar.copy(actwarm[:], actwarm[:])

            out_sb = const_pool.tile([128, NBLK], F32, tag="outsb")
            nc.vector.memset(out_sb[:], 0.0)

            # ---- streams. HWDGE dispatch costs ~620ns of ENGINE time per
            # dma_start, so dispatch count/placement matters as much as
            # bytes: tcc+pmat go as ONE dispatch each (early, small); the
            # W B-half dispatches are interleaved into the block loop one
            # window ahead so they never queue in front of ACT's texp
            # copies in program order.
            tcc_t = const_pool.tile([128, NWIN * 256], FP16, tag="tcc")
            nc.scalar.dma_start(out=tcc_t[:], in_=tcc[:])
            pt_t = const_pool.tile([128, NWIN * WSTR], PDT, tag="pt")
            nc.scalar.dma_start(out=pt_t[:], in_=pmat[:])

            hts = []
            wts = {}  # block m -> (tile, col offset)
            wbs = []  # deferred scalar-ring B-half dispatches
            for w in range(NWIN):
                b0 = bbase[w]
                nb = comp[w]
                ng = gw0[w + 1] - gw0[w]
                ht_t = const_pool.tile(
                    [128, ng * 64], FP16, tag=f"ht{w}", name=f"ht{w}"
                )
                nc.sync.dma_start(
                    out=ht_t[:], in_=hT[:, gw0[w] * 64 : gw0[w + 1] * 64]
                )
                hts.append(ht_t)
                ncols = min(G * 512, (b0 + nb) * 2048) - b0 * 2048
                half = (ncols // 2 + 511) // 512 * 512  # 512-col aligned split
                wt = w_pool.tile(
                    [128, nb * 2048], WDT, tag=f"wt{w}", name=f"wt{w}"
                )
                for m in range(b0, b0 + nb):
                    wts[m] = (wt, (m - b0) * 2048)
                nc.sync.dma_start(
                    out=wt[:, :half],
                    in_=wpk[:, b0 * 2048 : b0 * 2048 + half],
                )
                if w < 2:
                    nc.scalar.dma_start(
                        out=wt[:, half:ncols],
                        in_=wpk[:, b0 * 2048 + half : b0 * 2048 + ncols],
                    )
                    wbs.append(None)
                else:
                    wbs.append(
                        (wt, half, ncols, b0 * 2048 + half, b0 * 2048 + ncols)
                    )

            for m in range(NBLK):
                w_here = blk2win[m]
                if m == bbase[w_here] and w_here + 2 < NWIN and wbs[w_here + 2]:
                    wt2, h2, n2, s2, e2 = wbs[w_here + 2]
                    nc.scalar.dma_start(out=wt2[:, h2:n2], in_=wpk[:, s2:e2])
                    wbs[w_here + 2] = None
                nstrip = min(G, 4 * m + 4) - 4 * m
                w = blk2win[m]
                jloc = m - bbase[w]
                npart = 32 * nstrip

                texp = te_pool.tile(
                    [128, DIM], F32, space="PSUM", tag="texp", name=f"texp{m}"
                )
                nc.tensor.matmul(
                    out=texp[0:npart, :],
                    lhsT=pt_t[
                        :, w * WSTR + jloc * 128 : w * WSTR + jloc * 128 + npart
                    ],
                    rhs=tcc_t[:, w * 256 : (w + 1) * 256],
                    start=True,
                    stop=True,
                )
                texp_sb = te_sb_pool.tile(
                    [128, DIM], F32, tag="tesb", name=f"tesb{m}"
                )
                # fp8 dequant folded in: texp_sb = texp * (1/s_g per slot)
                nc.scalar.mul(
                    texp_sb[0:npart, :], texp[0:npart, :],
                    ds_t[0:npart, m : m + 1],
                )

                ups = u_pool.tile(
                    [128, DIM], F32, space="PSUM", tag="ups", name=f"ups{m}"
                )
                wtile, wcol = wts[m]
                hbase = (4 * m - gw0[w]) * 64
                for k in range(2):
                    for d in range(nstrip):
                        nc.tensor.matmul(
                            out=ups[32 * d : 32 * d + 32, :],
                            lhsT=hts[w][
                                :,
                                hbase + d * 64 + k * 32 : hbase + d * 64 + k * 32 + 32,
                            ],
                            rhs=wtile[
                                :,
                                wcol
                                + (d * 2 + k) * DIM : wcol
                                + (d * 2 + k + 1) * DIM,
                            ],
                            start=(k == 0),
                            stop=(k == 1),
                            tile_position=(0, 32 * d),
                            skip_group_check=True,
                        )

                # W8 scaling puts sc values ~3e-3 — fp16-safe, and the
                # 16-bit reduce input doubles DVE reduce throughput
                sc = sc_pool.tile(
                    [128, DIM], FP16 if w8 else F32, tag="sc", name=f"sc{m}"
                )
                nc.vector.tensor_tensor(
                    out=sc[0:npart, :],
                    in0=ups[0:npart, :],
                    in1=texp_sb[0:npart, :],
                    op=mybir.AluOpType.mult,
                )
                nc.vector.tensor_reduce(
                    out=out_sb[0:npart, m : m + 1],
                    in_=sc[0:npart, :],
                    axis=mybir.AxisListType.X,
                    op=mybir.AluOpType.add,
                )
                if m == 11 and NBLK > 12:
                    # early partial writeback: the final DMA then waits only
                    # on the last blocks' reduces
                    nc.sync.dma_start(
                        out=out[:, 0:12], in_=out_sb[:, 0:12]
                    )

            if NBLK > 12:
                nc.sync.dma_start(out=out[:, 12:], in_=out_sb[:, 12:])
            else:
                nc.sync.dma_start(out=out[:], in_=out_sb[:])

    return nc


_NC_CACHE = {}


def _get_nc(G, first):
    key = (G, first, W8)
    if key not in _NC_CACHE:
        _NC_CACHE[key] = _build_bass(G, first)
    return _NC_CACHE[key]


def _pack(h, r, t, ent_weight, rel_weight, first=3):
    """Group items by relation (chunks <=C), balance chunks across cores,
    pack each core's chunks into windows; host-gather h into transposed
    lhsT layout (hT) and t into compact per-window rows (tcc)."""
    order = np.argsort(r, kind="stable")
    rs = r[order]
    starts = np.flatnonzero(np.r_[True, rs[1:] != rs[:-1]])
    ends = np.r_[starts[1:], len(rs)]
    chunks = []  # (rel_id, item_positions)
    for s0, e0 in zip(starts, ends):
        rid = int(rs[s0])
        for c0 in range(s0, e0, C):
            chunks.append((rid, order[c0 : min(c0 + C, e0)]))
    chunks.sort(key=lambda x: -len(x[1]))

    per_core = [[] for _ in range(NCORES)]
    counts = [0] * NCORES
    items = [0] * NCORES
    for ch in chunks:
        k = min(range(NCORES), key=lambda q: (counts[q], items[q]))
        per_core[k].append(ch)
        counts[k] += 1
        items[k] += len(ch[1])

    G = max(counts)
    NBLK = (G + 3) // 4
    comp = _wcomp(NBLK, first)
    NWIN = len(comp)
    bbase = [0]
    for b in comp:
        bbase.append(bbase[-1] + b)
    quota = [min(4 * comp[wi], G - 4 * bbase[wi]) for wi in range(NWIN)]
    WSTR = _wstride(comp)

    wdt = ml_dtypes.float8_e3m4 if W8 else np.float16
    in_maps = []
    slot_maps = []
    for k in range(NCORES):
        # assign chunks to windows (greedy on item counts, sorted desc;
        # every window must fill to its quota and stay <=128 items)
        wins = [[] for _ in range(NWIN)]
        witems = [0] * NWIN
        for ch in per_core[k]:
            n = len(ch[1])
            cand = [
                wi
                for wi in range(NWIN)
                if len(wins[wi]) < quota[wi] and witems[wi] + n <= 128
            ]
            if not cand:  # fall through; the overflow assert fires below
                cand = [
                    wi for wi in range(NWIN) if len(wins[wi]) < quota[wi]
                ]
            wi = min(cand, key=lambda q: witems[q])
            wins[wi].append(ch)
            witems[wi] += n
        assert max(witems) <= 128, f"window overflow: {witems}"

        wpk = np.zeros((128, G * 512), dtype=wdt)
        hT = np.zeros((128, G * 64), dtype=np.float16)
        tcc = np.zeros((128, NWIN * 256), dtype=np.float16)
        pmat = np.zeros((128, NWIN * WSTR), dtype=np.float16)
        dscale = np.ones((128, NBLK), dtype=np.float32)
        slots = []
        positions = []
        for wi in range(NWIN):
            cpos = 0
            for gl, (rid, pos) in enumerate(wins[wi]):
                g = 4 * bbase[wi] + gl
                w3 = (
                    rel_weight[rid]
                    .reshape(2, 128, DIM)
                    .transpose(1, 0, 2)
                    .reshape(128, 512)
                )
                s = g * C + np.arange(len(pos))
                if W8:
                    sg = W8_TARGET / max(np.abs(w3).max(), 1e-30)
                    wpk[:, g * 512 : (g + 1) * 512] = (w3 * sg).astype(wdt)
                    dscale[s % 128, s // 128] = 1.0 / sg
                else:
                    wpk[:, g * 512 : (g + 1) * 512] = w3.astype(wdt)
                n = len(pos)
                he = ent_weight[h[pos]].astype(np.float16)  # [n, 256]
                hT[:, g * 64 : g * 64 + n] = he[:, 0:128].T
                hT[:, g * 64 + 32 : g * 64 + 32 + n] = he[:, 128:256].T
                cslot = cpos + np.arange(n)
                cpos += n
                tcc[cslot, wi * 256 : (wi + 1) * 256] = ent_weight[
                    t[pos]
                ].astype(np.float16)
                # window wi owns grid slots from 128*bbase[wi]
                local_s = s - 128 * bbase[wi]
                pmat[cslot, wi * WSTR + local_s] = 1.0
                slots.append(s)
                positions.append(pos)
        slots = np.concatenate(slots) if slots else np.zeros(0, np.int64)
        positions = (
            np.concatenate(positions) if positions else np.zeros(0, np.int64)
        )
        slot_maps.append((slots, positions))
        in_maps.append(
            {
                "wpk": wpk,
                "hT": hT,
                "tcc": tcc,
                "pmat": pmat.astype(
                    ml_dtypes.float8_e3m4 if W8 else np.float16
                ),
                "dscale": dscale,
            }
        )
    return in_maps, slot_maps, G


def _run(h, r, t, ent_weight, rel_weight, trace=False, mode=None):
    h = np.asarray(h).astype(np.int64)
    r = np.asarray(r).astype(np.int64)
    t = np.asarray(t).astype(np.int64)
    ent_weight = np.ascontiguousarray(np.asarray(ent_weight, dtype=np.float32))
    rel_weight = np.ascontiguousarray(np.asarray(rel_weight, dtype=np.float32))
    assert ent_weight.shape == (N_ENT, DIM)
    assert rel_weight.shape == (N_REL, DIM * DIM)

    first = 3
    in_maps, slot_maps, G = _pack(h, r, t, ent_weight, rel_weight, first=first)
    nc = _get_nc(G, first)
    res = run_bass_kernel_spmd(
        nc, in_maps, core_ids=list(range(NCORES)), trace=trace
    )
    scores = np.empty(h.shape[0], dtype=np.float32)
    for k in range(NCORES):
        o = res.results[k]["out"]
        slots, positions = slot_maps[k]
        scores[positions] = o[slots % 128, slots // 128]
    return scores, res


def kernel(h, r, t, ent_weight, rel_weight):
    scores, _ = _run(h, r, t, ent_weight, rel_weight, trace=False)
    return scores
